# revision 1
# baseline (speedup 1.0000x reference)
"""Trainium2 Bass kernel for nn_CanadarmJacob (space-arm Jacobian, bm=1 path).

Contract: kernel(**inputs) takes FULL inputs (com_list (512,256,3,7) f32,
link_pose_list (512,256,4,4,9) f32, bm scalar) and returns the FULL output
(512,256,6,7) f32. Internally shards samples across 8 NeuronCores (pure data
parallel), runs a Bass/Tile kernel per core, and gathers.

Math (reformulated from the reference):
  pos   = pose[:3, 3, :7]
  rot   = pose[:3, AXIS[a], a] with AXIS=[2,0,2,2,2,0,2], rot[:,4] *= -1
  del   = com - pos
  jacob = rot x del                          (per-act cross product)
  w_k   = sum_{a>=k} M_a del_a               (suffix mass-weighted cumsum)
  Hphi  = D_suf ⊙ rot + w x jacob            (D_suf = suffix inertia diags)
  S_cc' = sum_a M_a del_c del_c'             (6 unique entries)
  c     = (sum_a M_a com_a)/TM - BASE
  H_s   = TM(c c^T - |c|^2 I) + CD + (Sxx+Syy+Szz) I - S
  jsm   = SM_k jacob_k                       (SM = suffix masses)
  Hth   = Hphi - c x jsm
  A     = -inv(H_s)   (symmetric 3x3, via adjugate and ACT reciprocal)
  bot   = A @ Hth
  top   = -(1/TM) jsm + c x bot
  out   = concat(top, bot) rows
"""
import sys
import functools

if "/opt/trn_rl_repo" not in sys.path:
    sys.path.insert(0, "/opt/trn_rl_repo")

import numpy as np

# ---------------------------------------------------------------- constants
N_CORES = 8
P = 128          # SBUF partitions
J = 128          # samples per partition per core
S_CORE = P * J   # 16384 samples per core
N_ACT = 7

MASS = np.array([105.98, 105.98, 314.98, 279.2, 105.98, 105.98, 243.66], np.float64)
TM = float(MASS.sum() + 100000.0 + 243.66)
DIAGS = np.array([[12.19, 12.19, 3.061], [12.19, 12.19, 3.061], [15.41, 2094.71, 2103.19],
                  [9.522, 1966.28, 1966.28], [8.305, 3.061, 8.0386], [12.13, 12.13, 3.061],
                  [9.336, 44.41, 44.41]], np.float64)
D_SUF = np.cumsum(DIAGS[::-1], axis=0)[::-1]          # (7,3) suffix inertia diag
SM = np.cumsum(MASS[::-1])[::-1]                      # (7,) suffix mass
CD = DIAGS.sum(axis=0)                                # (3,)
_TF0 = np.array([[1, 0, 0, 0], [0, -1, 0, 0], [0, 0, 1.3, 6], [0, 0, 0, 1]], np.float64)
_COM0 = np.array([[1, 0, 0, 0], [0, 1, 0, 0], [0, 0, 1, 0.5], [0, 0, 0, 1]], np.float64)
BASE = (_TF0 @ _COM0)[:3, 3] * 243.66 / (100000.0 + 243.66)   # [0, 0, ~0.0162]

# consts row layout (replicated to all 128 partitions host-side)
#   [0:7] M, [7:14] SM, [14:35] D (c-major: D[c][a]), [35:38] CD
CONSTS = np.concatenate([MASS, SM, D_SUF.T.reshape(-1), CD]).astype(np.float32)
NCONST = CONSTS.shape[0]

# smalls tile row indices (each row is (128, J) f32)
SS_R, CSQ_R = 0, 1
CC = 2            # rows 2..4 diag (xx,yy,zz), 5..7 off (xy,xz,yz)
HS = 8            # rows 8..13: [xx,yy,zz,xy,xz,yz]
ADJ = 14          # rows 14..19: [a11,a22,a33,a12,a13,a23]
M1_R, M2_R = 20, 22
T0_R, T1_R, T2_R = 24, 25, 26
DET_R, RDN_R = 27, 28
NSMALL = 29


def _emit(nc, tc, ctx, dram):
    import concourse.bass as bass
    from concourse import mybir

    f32 = mybir.dt.float32
    OP = mybir.AluOpType
    AX = mybir.AxisListType
    V = nc.vector
    G = nc.gpsimd

    NCH = 4                       # pose DMA / early-stage pipeline chunks
    CJ = J // NCH

    pool = ctx.enter_context(tc.tile_pool(name="main", bufs=1))
    ppool = ctx.enter_context(tc.tile_pool(name="pp", bufs=NCH))

    consts = pool.tile([P, NCONST], f32)
    # pose chunks + outb rotate through the same NCH slots (sized by outb)
    poses = [ppool.tile([P, CJ, 144], f32, tag="px", name=f"pose{h}")
             for h in range(NCH)]
    com = pool.tile([P, J, 21], f32, tag="com")
    delb = pool.tile([P, 3, J, N_ACT], f32, tag="dj")
    mw = pool.tile([P, 3, J, N_ACT], f32)     # mdel, suffix-summed in place -> w
    jac = pool.tile([P, 3, J, N_ACT], f32)
    hphi = pool.tile([P, 3, J, N_ACT], f32)   # Hphi -> Htheta in place
    prod = pool.tile([P, 9, J, N_ACT], f32)   # 6 S-products + 3 mcom; later scratch
    cv = pool.tile([P, 3, J], f32)
    smalls = pool.tile([P, NSMALL, J], f32)
    outb = ppool.tile([P, J, 42], f32, tag="px")    # rotates onto pose slots
    red = pool.tile([P, 9, J], f32, tag="com")      # reuses com slot

    # All input DMAs on the sync HWDGE ring (FIFO). consts+com are small and
    # gate the first del chunk, so they go first; pose chunks follow and the
    # chunk-h compute pipelines behind them.
    nc.sync.dma_start(out=consts[:], in_=dram["consts"][:])
    nc.sync.dma_start(out=com[:], in_=dram["com"][:])
    for h in range(NCH):
        nc.sync.dma_start(out=poses[h][:],
                          in_=dram["pose"][:, h * CJ:(h + 1) * CJ, :])

    # handy views
    comR = com[:].rearrange("p j (c a) -> p c j a", c=3)          # (P,3,J,7)

    def bc(ap, shape):
        return ap.broadcast_to(shape)

    Mb = bc(consts[:, 0:7].unsqueeze(1).unsqueeze(2), (P, 3, CJ, N_ACT))
    SMb = bc(consts[:, 7:14].unsqueeze(1).unsqueeze(2), (P, 3, J, N_ACT))
    Db = bc(consts[:, 14:35].rearrange("p (c a) -> p c a", c=3).unsqueeze(2),
            (P, 3, J, N_ACT))
    CDb = bc(consts[:, 35:38].unsqueeze(2), (P, 3, J))

    # early stages per pose chunk, pipelined behind the chunk DMAs
    rots = []
    for h in range(NCH):
        js = slice(h * CJ, (h + 1) * CJ)
        poseR = poses[h][:].rearrange("p j (r k) -> p r j k", r=4)[:, 0:3]
        # rot = axis-col-2 block patched in place: acts 1,5 from axis col 0,
        # act 4 sign-flipped. No gather copy needed.  poseR: (P,3,CJ,36)
        V.tensor_copy(out=poseR[:, :, :, 19:24:4], in_=poseR[:, :, :, 1:6:4])
        V.tensor_scalar_mul(poseR[:, :, :, 22], poseR[:, :, :, 22], -1.0)
        rot = poseR[:, :, :, 18:25]
        rots.append(rot)
        posV = poseR[:, :, :, 27:34]
        dl = delb[:, :, js]
        V.tensor_tensor(out=dl, in0=comR[:, :, js], in1=posV, op=OP.subtract)
        V.tensor_tensor(out=mw[:, :, js], in0=Mb, in1=dl, op=OP.mult)
        for k, (i, j) in enumerate([(0, 0), (1, 1), (2, 2), (0, 1), (0, 2), (1, 2)]):
            V.tensor_tensor(out=prod[:, k, js], in0=mw[:, i, js], in1=delb[:, j, js],
                            op=OP.mult)
        V.tensor_tensor(out=prod[:, 6:9, js], in0=Mb, in1=comR[:, :, js], op=OP.mult)
        for cx in range(3):
            y, z = (cx + 1) % 3, (cx + 2) % 3
            V.tensor_tensor(out=hphi[:, y, js], in0=rot[:, y], in1=delb[:, z, js],
                            op=OP.mult)
            V.tensor_tensor(out=hphi[:, z, js], in0=rot[:, z], in1=delb[:, y, js],
                            op=OP.mult)
            V.tensor_tensor(out=jac[:, cx, js], in0=hphi[:, y, js],
                            in1=hphi[:, z, js], op=OP.subtract)
        # act-reduction of the 9 product rows for this chunk
        V.tensor_reduce(out=red[:, :, js], in_=prod[:, :, js], axis=AX.X, op=OP.add)

    tu = prod[:, 0:3]
    tv = prod[:, 3:6]

    # c = scom/TM - BASE   (BASE is [0,0,bz])
    V.tensor_scalar(out=cv[:, 0:2], in0=red[:, 6:8], scalar1=1.0 / TM, scalar2=None,
                    op0=OP.mult)
    V.tensor_scalar(out=cv[:, 2], in0=red[:, 8], scalar1=1.0 / TM,
                    scalar2=float(BASE[2]), op0=OP.mult, op1=OP.subtract)

    # suffix cumsum over acts in place: mw becomes w
    for k in range(5, -1, -1):
        V.tensor_tensor(out=mw[:, :, :, k], in0=mw[:, :, :, k], in1=mw[:, :, :, k + 1],
                        op=OP.add)

    # w2 = w - SM∘c folds the former  Htheta = Hphi - c x jsm  stage into the
    # Hphi cross product:  Htheta = D⊙rot + (w - SM∘c) x jacob
    cvb3 = bc(cv[:].unsqueeze(3), (P, 3, J, N_ACT))
    V.tensor_tensor(out=tu[:], in0=SMb, in1=cvb3, op=OP.mult)
    V.tensor_tensor(out=mw[:], in0=mw[:], in1=tu[:], op=OP.subtract)

    # jsm = SM * jacob (reuses delb slot via tag)
    jsm = delb  # overwritten after last delb read (jacob products)
    V.tensor_tensor(out=jsm[:], in0=SMb, in1=jac[:], op=OP.mult)

    # Htheta = D*rot + w2 x jacob  (written into hphi)
    for cx in range(3):
        y, z = (cx + 1) % 3, (cx + 2) % 3
        V.tensor_tensor(out=tu[:, cx], in0=mw[:, y], in1=jac[:, z], op=OP.mult)
        V.tensor_tensor(out=tv[:, cx], in0=mw[:, z], in1=jac[:, y], op=OP.mult)
        V.tensor_tensor(out=hphi[:, cx], in0=tu[:, cx], in1=tv[:, cx], op=OP.subtract)
    DbC = bc(consts[:, 14:35].rearrange("p (c a) -> p c a", c=3).unsqueeze(2),
             (P, 3, CJ, N_ACT))
    for h in range(NCH):
        js = slice(h * CJ, (h + 1) * CJ)
        V.tensor_tensor(out=tu[:, :, js], in0=rots[h], in1=DbC, op=OP.mult)
        V.tensor_tensor(out=hphi[:, :, js], in0=hphi[:, :, js], in1=tu[:, :, js],
                        op=OP.add)

    # cc products and |c|^2, SS
    V.tensor_tensor(out=smalls[:, CC:CC + 3], in0=cv[:], in1=cv[:], op=OP.mult)
    for k, (i, j) in enumerate([(0, 1), (0, 2), (1, 2)]):
        V.tensor_tensor(out=smalls[:, CC + 3 + k], in0=cv[:, i], in1=cv[:, j],
                        op=OP.mult)
    V.tensor_reduce(out=smalls[:, SS_R], in_=red[:, 0:3].transpose([0, 2, 1]),
                    axis=AX.X, op=OP.add)
    V.tensor_reduce(out=smalls[:, CSQ_R], in_=smalls[:, CC:CC + 3].transpose([0, 2, 1]),
                    axis=AX.X, op=OP.add)

    csq_b = bc(smalls[:, CSQ_R].unsqueeze(1), (P, 3, J))
    ss_b = bc(smalls[:, SS_R].unsqueeze(1), (P, 3, J))

    # H_s diag rows HS..HS+2 ; off rows HS+3..HS+5
    a1 = smalls[:, M1_R:M1_R + 2]  # scratch pair rows (reused a lot below)
    V.tensor_tensor(out=smalls[:, T0_R:T0_R + 3], in0=smalls[:, CC:CC + 3], in1=csq_b,
                    op=OP.subtract)
    V.tensor_tensor(out=smalls[:, HS:HS + 3], in0=ss_b, in1=red[:, 0:3], op=OP.subtract)
    nc.vector.scalar_tensor_tensor(out=smalls[:, HS:HS + 3], in0=smalls[:, T0_R:T0_R + 3],
                                   scalar=TM, in1=smalls[:, HS:HS + 3],
                                   op0=OP.mult, op1=OP.add)
    V.tensor_tensor(out=smalls[:, HS:HS + 3], in0=smalls[:, HS:HS + 3], in1=CDb,
                    op=OP.add)
    nc.vector.scalar_tensor_tensor(out=smalls[:, HS + 3:HS + 6],
                                   in0=smalls[:, CC + 3:CC + 6], scalar=TM,
                                   in1=red[:, 3:6], op0=OP.mult, op1=OP.subtract)

    # adjugate (batched pairs via reversed/broadcast row views)
    h = lambda i: smalls[:, HS + i]
    hpair = lambda a, b: smalls[:, HS + a: (HS + b - 1 if b < a else HS + b + 1): (1 if b > a else -1)]
    b2 = lambda ap: bc(ap.unsqueeze(1), (P, 2, J))
    # a11 = h1 h2 - h5^2 ; a22 = h0 h2 - h4^2
    V.tensor_tensor(out=smalls[:, M1_R:M1_R + 2], in0=hpair(1, 0), in1=b2(h(2)), op=OP.mult)
    V.tensor_tensor(out=smalls[:, M2_R:M2_R + 2], in0=hpair(5, 4), in1=hpair(5, 4), op=OP.mult)
    V.tensor_tensor(out=smalls[:, ADJ:ADJ + 2], in0=smalls[:, M1_R:M1_R + 2],
                    in1=smalls[:, M2_R:M2_R + 2], op=OP.subtract)
    # a33 = h0 h1 - h3^2
    V.tensor_tensor(out=smalls[:, T0_R], in0=h(0), in1=h(1), op=OP.mult)
    V.tensor_tensor(out=smalls[:, T1_R], in0=h(3), in1=h(3), op=OP.mult)
    V.tensor_tensor(out=smalls[:, ADJ + 2], in0=smalls[:, T0_R], in1=smalls[:, T1_R],
                    op=OP.subtract)
    # a12 = h4 h5 - h3 h2 ; a13 = h3 h5 - h4 h1
    V.tensor_tensor(out=smalls[:, M1_R:M1_R + 2], in0=hpair(4, 3), in1=b2(h(5)), op=OP.mult)
    V.tensor_tensor(out=smalls[:, M2_R:M2_R + 2], in0=hpair(3, 4), in1=hpair(2, 1), op=OP.mult)
    V.tensor_tensor(out=smalls[:, ADJ + 3:ADJ + 5], in0=smalls[:, M1_R:M1_R + 2],
                    in1=smalls[:, M2_R:M2_R + 2], op=OP.subtract)
    # a23 = h3 h4 - h0 h5
    V.tensor_tensor(out=smalls[:, T0_R], in0=h(3), in1=h(4), op=OP.mult)
    V.tensor_tensor(out=smalls[:, T1_R], in0=h(0), in1=h(5), op=OP.mult)
    V.tensor_tensor(out=smalls[:, ADJ + 5], in0=smalls[:, T0_R], in1=smalls[:, T1_R],
                    op=OP.subtract)

    # det = h0 a11 + h3 a12 + h4 a13 ; A = adj * (-1/det)
    V.tensor_tensor(out=smalls[:, T0_R], in0=h(0), in1=smalls[:, ADJ], op=OP.mult)
    V.tensor_tensor(out=smalls[:, T1_R], in0=h(3), in1=smalls[:, ADJ + 3], op=OP.mult)
    V.tensor_tensor(out=smalls[:, T2_R], in0=h(4), in1=smalls[:, ADJ + 4], op=OP.mult)
    V.tensor_tensor(out=smalls[:, DET_R], in0=smalls[:, T0_R], in1=smalls[:, T1_R],
                    op=OP.add)
    V.tensor_tensor(out=smalls[:, DET_R], in0=smalls[:, DET_R], in1=smalls[:, T2_R],
                    op=OP.add)
    V.reciprocal(out=smalls[:, RDN_R], in_=smalls[:, DET_R])
    rdn_b = bc(smalls[:, RDN_R].unsqueeze(1), (P, 6, J))
    nc.vector.scalar_tensor_tensor(out=smalls[:, ADJ:ADJ + 6],
                                   in0=smalls[:, ADJ:ADJ + 6], scalar=-1.0,
                                   in1=rdn_b, op0=OP.mult, op1=OP.mult)

    # bot = A @ Htheta -> outb cols 21..41 ; top = -(1/TM) jsm + c x bot ->
    # cols 0..20.  Done in two j-halves so the first half's output DMA
    # (168B-contiguous runs) overlaps the second half's compute.
    Arows = [[0, 3, 4], [3, 1, 5], [4, 5, 2]]
    JH = J // 2
    for g in range(2):
        gs = slice(g * JH, (g + 1) * JH)
        cvb = lambda i: bc(cv[:, i, gs].unsqueeze(2), (P, JH, N_ACT))
        Ab = lambda r: bc(smalls[:, ADJ + r, gs].unsqueeze(2), (P, JH, N_ACT))
        bot = lambda c: outb[:, gs, 21 + 7 * c: 28 + 7 * c]
        for oc in range(3):
            r0, r1, r2 = Arows[oc]
            V.tensor_tensor(out=tu[:, 0, gs], in0=Ab(r0), in1=hphi[:, 0, gs], op=OP.mult)
            V.tensor_tensor(out=tu[:, 1, gs], in0=Ab(r1), in1=hphi[:, 1, gs], op=OP.mult)
            V.tensor_tensor(out=tu[:, 2, gs], in0=tu[:, 0, gs], in1=tu[:, 1, gs], op=OP.add)
            V.tensor_tensor(out=tu[:, 0, gs], in0=Ab(r2), in1=hphi[:, 2, gs], op=OP.mult)
            V.tensor_tensor(out=bot(oc), in0=tu[:, 2, gs], in1=tu[:, 0, gs], op=OP.add)
        for cx in range(3):
            y, z = (cx + 1) % 3, (cx + 2) % 3
            V.tensor_tensor(out=tu[:, cx, gs], in0=cvb(y), in1=bot(z), op=OP.mult)
            nc.vector.scalar_tensor_tensor(out=tv[:, cx, gs], in0=jsm[:, cx, gs],
                                           scalar=-1.0 / TM, in1=tu[:, cx, gs],
                                           op0=OP.mult, op1=OP.add)
            V.tensor_tensor(out=tu[:, cx, gs], in0=cvb(z), in1=bot(y), op=OP.mult)
            V.tensor_tensor(out=outb[:, gs, 7 * cx: 7 * cx + 7], in0=tv[:, cx, gs],
                            in1=tu[:, cx, gs], op=OP.subtract)
        nc.sync.dma_start(out=dram["out"][:, gs], in_=outb[:, gs])


@functools.lru_cache(maxsize=1)
def _program():
    from contextlib import ExitStack
    import concourse.bacc as bacc
    import concourse.tile as tile
    from concourse import mybir

    f32 = mybir.dt.float32
    nc = bacc.Bacc("TRN2", target_bir_lowering=False, debug=False)
    dram = {
        "com": nc.dram_tensor("com", [P, J, 21], f32, kind="ExternalInput"),
        "pose": nc.dram_tensor("pose", [P, J, 144], f32, kind="ExternalInput"),
        "consts": nc.dram_tensor("consts", [P, NCONST], f32, kind="ExternalInput"),
        "out": nc.dram_tensor("out", [P, J, 42], f32, kind="ExternalOutput"),
    }
    with tile.TileContext(nc) as tc:
        with ExitStack() as ctx:
            _emit(nc, tc, ctx, dram)
    nc.compile()
    return nc


def _kernel_bm0(com, pose):
    # bm=0 path (not exercised by the shipped setup_inputs; numpy fallback)
    rot = pose[:, :, :3, 2, :N_ACT].copy()
    rot[..., 1] = pose[:, :, :3, 0, 1]
    rot[..., 5] = pose[:, :, :3, 0, 5]
    rot[..., 4] *= -1.0
    delp = pose[:, :, :3, 3, -2][..., None] - pose[:, :, :3, 3, :N_ACT]
    jt = np.cross(rot, delp, axis=2)
    return np.concatenate([jt, rot], axis=2).astype(np.float32)


def kernel(com_list, link_pose_list, bm):
    com_list = np.ascontiguousarray(com_list, dtype=np.float32)
    link_pose_list = np.ascontiguousarray(link_pose_list, dtype=np.float32)
    if not int(bm):
        return _kernel_bm0(com_list, link_pose_list)

    from concourse.bass_utils import run_bass_kernel_spmd

    nc = _program()
    com_flat = com_list.reshape(N_CORES, P, J, 21)
    pose_flat = link_pose_list.reshape(N_CORES, P, J, 144)
    consts = np.broadcast_to(CONSTS, (P, NCONST)).copy()
    in_maps = [
        {"com": com_flat[k], "pose": pose_flat[k], "consts": consts}
        for k in range(N_CORES)
    ]
    res = run_bass_kernel_spmd(nc, in_maps, core_ids=list(range(N_CORES)))
    out = np.stack([res.results[k]["out"] for k in range(N_CORES)])
    return out.reshape(512, 256, 6, 7)



# revision 4
# speedup vs baseline: 1.3726x; 1.3726x over previous
"""Trainium2 Bass kernel for nn_CanadarmJacob (space-arm Jacobian, bm=1 path).

Contract: kernel(**inputs) takes FULL inputs (com_list (512,256,3,7) f32,
link_pose_list (512,256,4,4,9) f32, bm scalar) and returns the FULL output
(512,256,6,7) f32. Internally shards samples across 8 NeuronCores (pure data
parallel), runs a Bass/Tile kernel per core, and gathers.

v2 design: J-innermost bf16 layout (P, row, act, J).
  - All bulk elementwise ops are bf16 tensor_tensor at the DVE 2x perf mode
    (innermost step 1, even runs). Per-sample scalars (c, A rows, 1/det)
    broadcast over the act dim with innermost step 1, staying at 2x.
  - ScalarE (Activation engine) does all input marshaling: transpose+cast of
    com/pose columns into the T layout, plus constant-tile materialization
    and the output cast/transpose back to (J, 42) fp32.
  - Act-dim reductions are shifted-add trees at 2x instead of 1x tensor_reduce.
  - Cross products use 5-row tiles (rows 3,4 duplicate 0,1) so rot[c+1] etc.
    are contiguous 3-row views -> 3 big TT ops instead of 9 small ones.
  - w2 = w - SM*c folds into the suffix cumsum: w2 = sufcum(mdel - M*c).
  - Pose DMA fetches only rows r<3 (432 of 576 B/sample).

Math (same reformulation as v1):
  pos   = pose[:3, 3, :7];  rot = pose[:3, AXIS[a], a], rot[:,4] *= -1
  del   = com - pos;  jacob = rot x del
  w2    = sufcum_a(M*del - M*c);  Hth = D_suf (.) rot + w2 x jacob
  S_cc' = sum_a M_a del_c del_c';  c = (sum_a M_a com_a)/TM - BASE
  H_s   = TM(c c^T - |c|^2 I) + CD I + SS I - S
  A     = -inv(H_s);  bot = A @ Hth
  top   = -(SM/TM) (.) jacob + c x bot
"""
import sys
import functools

if "/opt/trn_rl_repo" not in sys.path:
    sys.path.insert(0, "/opt/trn_rl_repo")

import numpy as np

# ---------------------------------------------------------------- constants
N_CORES = 8
P = 128          # SBUF partitions
J = 128          # samples per partition per core
NCH = 4          # pose DMA quarters
CJ = J // NCH
JH = J // 2
N_ACT = 7

MASS = np.array([105.98, 105.98, 314.98, 279.2, 105.98, 105.98, 243.66], np.float64)
TM = float(MASS.sum() + 100000.0 + 243.66)
DIAGS = np.array([[12.19, 12.19, 3.061], [12.19, 12.19, 3.061], [15.41, 2094.71, 2103.19],
                  [9.522, 1966.28, 1966.28], [8.305, 3.061, 8.0386], [12.13, 12.13, 3.061],
                  [9.336, 44.41, 44.41]], np.float64)
D_SUF = np.cumsum(DIAGS[::-1], axis=0)[::-1]          # (7,3) suffix inertia diag
SM = np.cumsum(MASS[::-1])[::-1]                      # (7,) suffix mass
CD = DIAGS.sum(axis=0)                                # (3,)
_TF0 = np.array([[1, 0, 0, 0], [0, -1, 0, 0], [0, 0, 1.3, 6], [0, 0, 0, 1]], np.float64)
_COM0 = np.array([[1, 0, 0, 0], [0, 1, 0, 0], [0, 0, 1, 0.5], [0, 0, 0, 1]], np.float64)
BASE = (_TF0 @ _COM0)[:3, 3] * 243.66 / (100000.0 + 243.66)   # [0, 0, ~0.0162]

# consts row layout (replicated to all 128 partitions host-side)
#   [0:7] M, [7:14] SM/TM, [14:35] D (c-major: D[c][a]), [35:38] CD
CONSTS = np.concatenate([MASS, SM / TM, D_SUF.T.reshape(-1), CD]).astype(np.float32)
NCONST = CONSTS.shape[0]

# smalls tile row indices (each row is (128, J) f32)
SS_R, CSQ_R = 0, 1
CC = 2            # rows 2..4 diag (xx,yy,zz), 5..7 off (xy,xz,yz)
HS = 8            # rows 8..13: [xx,yy,zz,xy,xz,yz]
M1_R, M2_R = 14, 16
T0_R, T1_R, T2_R = 18, 19, 20
DET_R, RDN_R = 21, 22
NSMALL = 23


def _emit(nc, tc, ctx, dram):
    from concourse import mybir

    f32 = mybir.dt.float32
    bf16 = mybir.dt.bfloat16
    OP = mybir.AluOpType
    V = nc.vector
    S = nc.scalar

    pool = ctx.enter_context(tc.tile_pool(name="main", bufs=1))
    ppool = ctx.enter_context(tc.tile_pool(name="pp", bufs=2))

    consts = pool.tile([P, NCONST], f32)
    com = pool.tile([P, J, 21], f32, tag="io")
    poses = [ppool.tile([P, CJ, 108], f32, tag="px", name=f"pose{h}")
             for h in range(NCH)]

    # T-layout tiles, J innermost (bf16 unless noted)
    comT = pool.tile([P, 3, N_ACT, J], bf16)
    delT5 = pool.tile([P, 5, N_ACT, J], bf16, tag="five1")  # 0:3 del; 3,4 dup 0,1
    rotT5 = pool.tile([P, 5, N_ACT, J], bf16)
    mdelT = pool.tile([P, 3, N_ACT, J], bf16, tag="m37")
    prodT = pool.tile([P, 9, N_ACT, J], bf16, tag="big9")  # 6 S + 3 mcom rows
    treeT = pool.tile([P, 9, 3, J], bf16, tag="tmp9")  # reduction tree scratch
    red9 = pool.tile([P, 9, J], f32)            # act-sums: S(6), scom(3)
    w2T5 = pool.tile([P, 5, N_ACT, J], bf16)
    jacT5 = pool.tile([P, 5, N_ACT, J], bf16)
    HthT = pool.tile([P, 3, N_ACT, J], bf16)
    jsmT = pool.tile([P, 3, N_ACT, J], bf16)
    ta = pool.tile([P, 3, N_ACT, J], bf16)
    tb = pool.tile([P, 3, N_ACT, J], bf16)
    cvT = pool.tile([P, 3, J], f32)
    cv5T = pool.tile([P, 5, J], bf16)
    smalls = pool.tile([P, NSMALL, J], f32)
    A9 = pool.tile([P, 9, J], f32, tag="tmp9")  # adjugate, row-major 3x3
    A9b = pool.tile([P, 9, J], bf16)
    P9 = pool.tile([P, 9, N_ACT, J], bf16, tag="big9")
    botT5 = pool.tile([P, 5, N_ACT, J], bf16, tag="five1")
    topT = pool.tile([P, 3, N_ACT, J], bf16, tag="m37")
    outb = pool.tile([P, J, 42], f32, tag="io")

    # Tag-based overlap: HthT reuses comT's slot region? No -- keep separate
    # tags; pools pack by declaration. (SBUF fits: ~200KB/partition.)

    # ---------------- input DMAs (sync ring, FIFO order)
    nc.sync.dma_start(out=consts[:], in_=dram["consts"][:])
    nc.sync.dma_start(out=com[:], in_=dram["com"][:])
    for h in range(NCH):
        js = slice(h * CJ, (h + 1) * CJ)
        # rows r<3 of each 4x4x9 pose = first 108 of 144 floats per sample
        nc.sync.dma_start(out=poses[h][:], in_=dram["pose"][:, js, 0:108])

    def bc(ap, shape):
        return ap.broadcast_to(shape)

    # ---------------- ScalarE: materialize bf16 const tiles
    Mt = pool.tile([P, N_ACT, J], bf16)
    SMTt = pool.tile([P, N_ACT, J], bf16)
    Dt = pool.tile([P, 3, N_ACT, J], bf16)
    S.copy(out=Mt[:], in_=bc(consts[:, 0:7].unsqueeze(2), (P, N_ACT, J)))
    S.copy(out=SMTt[:], in_=bc(consts[:, 7:14].unsqueeze(2), (P, N_ACT, J)))
    S.copy(out=Dt[:], in_=bc(
        consts[:, 14:35].rearrange("p (c a) -> p c a", c=3).unsqueeze(3),
        (P, 3, N_ACT, J)))
    Mb = bc(Mt[:].unsqueeze(1), (P, 3, N_ACT, J))
    SMTb = bc(SMTt[:].unsqueeze(1), (P, 3, N_ACT, J))

    # per-quarter marshal (ScalarE) + early compute (DVE)
    for h in range(NCH):
        js = slice(h * CJ, (h + 1) * CJ)
        # pose chunk viewed (p, j, r, k, l)
        pv = poses[h][:].rearrange("p j (r k l) -> p r k l j", r=3, k=4)
        # comT chunk: (p, c, a, j)
        S.copy(out=comT[:, :, :, js],
               in_=com[:, js, :].rearrange("p j (c a) -> p c a j", c=3))
        # pos -> delT5 rows 0:3 (holds pos until the sub)
        S.copy(out=delT5[:, 0:3, :, js], in_=pv[:, :, 3, 0:7])
        # rot main: axis col 2
        S.copy(out=rotT5[:, 0:3, :, js], in_=pv[:, :, 2, 0:7])
        # patch acts 1,5 from axis col 0
        S.copy(out=rotT5[:, 0:3, 1:6:4, js], in_=pv[:, :, 0, 1:6:4])
        # sign-flip act 4
        S.mul(rotT5[:, 0:3, 4, js], rotT5[:, 0:3, 4, js], -1.0)

        # --- DVE early stage (quarter)
        # del = com - pos  (in place over the pos copy)
        V.tensor_tensor(out=delT5[:, 0:3, :, js], in0=comT[:, :, :, js],
                        in1=delT5[:, 0:3, :, js], op=OP.subtract)
        # mdel = M * del
        V.tensor_tensor(out=mdelT[:, :, :, js], in0=Mb[:, :, :, js],
                        in1=delT5[:, 0:3, :, js], op=OP.mult)
        # S products: rows [S00,S11,S22, S01,S02, S12]
        V.tensor_tensor(out=prodT[:, 0:3, :, js], in0=mdelT[:, :, :, js],
                        in1=delT5[:, 0:3, :, js], op=OP.mult)
        V.tensor_tensor(out=prodT[:, 3:5, :, js],
                        in0=bc(mdelT[:, 0:1, :, js], (P, 2, N_ACT, CJ)),
                        in1=delT5[:, 1:3, :, js], op=OP.mult)
        V.tensor_tensor(out=prodT[:, 5, :, js], in0=mdelT[:, 1, :, js],
                        in1=delT5[:, 2, :, js], op=OP.mult)
        # mcom rows 6:9
        V.tensor_tensor(out=prodT[:, 6:9, :, js], in0=Mb[:, :, :, js],
                        in1=comT[:, :, :, js], op=OP.mult)
        # act-sum tree: sum7 = ((x0+x4)+(x1+x5)) + ((x2+x6)+x3)
        V.tensor_tensor(out=treeT[:, :, :, js], in0=prodT[:, :, 0:3, js],
                        in1=prodT[:, :, 4:7, js], op=OP.add)
        V.tensor_tensor(out=treeT[:, :, 0, js], in0=treeT[:, :, 0, js],
                        in1=treeT[:, :, 1, js], op=OP.add)
        V.tensor_tensor(out=treeT[:, :, 2, js], in0=treeT[:, :, 2, js],
                        in1=prodT[:, :, 3, js], op=OP.add)
        V.tensor_tensor(out=red9[:, :, js], in0=treeT[:, :, 0, js],
                        in1=treeT[:, :, 2, js], op=OP.add)

        # ---- mid stage per half, interleaved after quarters 1 and 3
        if h % 2 == 1:
            g = h // 2
            jh = slice(g * JH, (g + 1) * JH)
            # c = scom/TM - BASE  (fp32)
            V.tensor_scalar(out=cvT[:, 0:2, jh], in0=red9[:, 6:8, jh],
                            scalar1=1.0 / TM, scalar2=None, op0=OP.mult)
            V.tensor_scalar(out=cvT[:, 2, jh], in0=red9[:, 8, jh],
                            scalar1=1.0 / TM, scalar2=float(BASE[2]),
                            op0=OP.mult, op1=OP.subtract)
            # cv5 bf16 + dup rows
            V.tensor_copy(out=cv5T[:, 0:3, jh], in_=cvT[:, :, jh])
            V.tensor_copy(out=cv5T[:, 3:5, jh], in_=cv5T[:, 0:2, jh])
            # dups for shifted-row cross products
            V.tensor_copy(out=delT5[:, 3:5, :, jh], in_=delT5[:, 0:2, :, jh])
            V.tensor_copy(out=rotT5[:, 3:5, :, jh], in_=rotT5[:, 0:2, :, jh])
            cvb = bc(cv5T[:, 0:3, jh].unsqueeze(2), (P, 3, N_ACT, JH))
            # w2 = sufcum(mdel - M*c)
            V.tensor_tensor(out=ta[:, :, :, jh], in0=Mb[:, :, :, jh], in1=cvb,
                            op=OP.mult)
            V.tensor_tensor(out=w2T5[:, 0:3, :, jh], in0=mdelT[:, :, :, jh],
                            in1=ta[:, :, :, jh], op=OP.subtract)
            for k in range(5, -1, -1):
                V.tensor_tensor(out=w2T5[:, 0:3, k, jh],
                                in0=w2T5[:, 0:3, k, jh],
                                in1=w2T5[:, 0:3, k + 1, jh], op=OP.add)
            V.tensor_copy(out=w2T5[:, 3:5, :, jh], in_=w2T5[:, 0:2, :, jh])
            # jac = rot x del
            V.tensor_tensor(out=ta[:, :, :, jh], in0=rotT5[:, 1:4, :, jh],
                            in1=delT5[:, 2:5, :, jh], op=OP.mult)
            V.tensor_tensor(out=tb[:, :, :, jh], in0=rotT5[:, 2:5, :, jh],
                            in1=delT5[:, 1:4, :, jh], op=OP.mult)
            V.tensor_tensor(out=jacT5[:, 0:3, :, jh], in0=ta[:, :, :, jh],
                            in1=tb[:, :, :, jh], op=OP.subtract)
            V.tensor_copy(out=jacT5[:, 3:5, :, jh], in_=jacT5[:, 0:2, :, jh])
            # Hth = D*rot + w2 x jac
            V.tensor_tensor(out=ta[:, :, :, jh], in0=w2T5[:, 1:4, :, jh],
                            in1=jacT5[:, 2:5, :, jh], op=OP.mult)
            V.tensor_tensor(out=tb[:, :, :, jh], in0=w2T5[:, 2:5, :, jh],
                            in1=jacT5[:, 1:4, :, jh], op=OP.mult)
            V.tensor_tensor(out=HthT[:, :, :, jh], in0=ta[:, :, :, jh],
                            in1=tb[:, :, :, jh], op=OP.subtract)
            V.tensor_tensor(out=ta[:, :, :, jh], in0=Dt[:, :, :, jh],
                            in1=rotT5[:, 0:3, :, jh], op=OP.mult)
            V.tensor_tensor(out=HthT[:, :, :, jh], in0=HthT[:, :, :, jh],
                            in1=ta[:, :, :, jh], op=OP.add)
            # jsm = (SM/TM) * jac
            V.tensor_tensor(out=jsmT[:, :, :, jh], in0=SMTb[:, :, :, jh],
                            in1=jacT5[:, 0:3, :, jh], op=OP.mult)

    # ---------------- smalls chain (fp32, full J) -> A9b (bf16)
    # cc products and |c|^2, SS
    V.tensor_tensor(out=smalls[:, CC:CC + 3], in0=cvT[:], in1=cvT[:], op=OP.mult)
    for k, (i, j) in enumerate([(0, 1), (0, 2), (1, 2)]):
        V.tensor_tensor(out=smalls[:, CC + 3 + k], in0=cvT[:, i], in1=cvT[:, j],
                        op=OP.mult)
    V.tensor_tensor(out=smalls[:, T0_R], in0=red9[:, 0], in1=red9[:, 1], op=OP.add)
    V.tensor_tensor(out=smalls[:, SS_R], in0=smalls[:, T0_R], in1=red9[:, 2],
                    op=OP.add)
    V.tensor_tensor(out=smalls[:, T0_R], in0=smalls[:, CC], in1=smalls[:, CC + 1],
                    op=OP.add)
    V.tensor_tensor(out=smalls[:, CSQ_R], in0=smalls[:, T0_R], in1=smalls[:, CC + 2],
                    op=OP.add)

    csq_b = bc(smalls[:, CSQ_R].unsqueeze(1), (P, 3, J))
    ss_b = bc(smalls[:, SS_R].unsqueeze(1), (P, 3, J))
    CDb = bc(consts[:, 35:38].unsqueeze(2), (P, 3, J))

    # H_s diag rows HS..HS+2 ; off rows HS+3..HS+5
    V.tensor_tensor(out=smalls[:, T0_R:T0_R + 3], in0=smalls[:, CC:CC + 3], in1=csq_b,
                    op=OP.subtract)
    V.tensor_tensor(out=smalls[:, HS:HS + 3], in0=ss_b, in1=red9[:, 0:3], op=OP.subtract)
    V.scalar_tensor_tensor(out=smalls[:, HS:HS + 3], in0=smalls[:, T0_R:T0_R + 3],
                           scalar=TM, in1=smalls[:, HS:HS + 3],
                           op0=OP.mult, op1=OP.add)
    V.tensor_tensor(out=smalls[:, HS:HS + 3], in0=smalls[:, HS:HS + 3], in1=CDb,
                    op=OP.add)
    V.scalar_tensor_tensor(out=smalls[:, HS + 3:HS + 6],
                           in0=smalls[:, CC + 3:CC + 6], scalar=TM,
                           in1=red9[:, 3:6], op0=OP.mult, op1=OP.subtract)

    # adjugate into A9 (row-major [c00,c01,c02, c10,c11,c12, c20,c21,c22];
    # unique cofactors first: c00->r0, c11->r4, c22->r8, c01->r1, c02->r2, c12->r5)
    h_ = lambda i: smalls[:, HS + i]
    hpair = lambda a, b: smalls[:, HS + a: (HS + b - 1 if b < a else HS + b + 1): (1 if b > a else -1)]
    b2 = lambda ap: bc(ap.unsqueeze(1), (P, 2, J))
    # c00 = h1 h2 - h5^2 ; c11 = h0 h2 - h4^2   -> rows {0,4}
    V.tensor_tensor(out=smalls[:, M1_R:M1_R + 2], in0=hpair(1, 0), in1=b2(h_(2)), op=OP.mult)
    V.tensor_tensor(out=smalls[:, M2_R:M2_R + 2], in0=hpair(5, 4), in1=hpair(5, 4), op=OP.mult)
    V.tensor_tensor(out=A9[:, 0:5:4], in0=smalls[:, M1_R:M1_R + 2],
                    in1=smalls[:, M2_R:M2_R + 2], op=OP.subtract)
    # c22 = h0 h1 - h3^2  -> row 8
    V.tensor_tensor(out=smalls[:, T0_R], in0=h_(0), in1=h_(1), op=OP.mult)
    V.tensor_tensor(out=smalls[:, T1_R], in0=h_(3), in1=h_(3), op=OP.mult)
    V.tensor_tensor(out=A9[:, 8], in0=smalls[:, T0_R], in1=smalls[:, T1_R],
                    op=OP.subtract)
    # c01 = h4 h5 - h3 h2 ; c02 = h3 h5 - h4 h1  -> rows {1,2}
    V.tensor_tensor(out=smalls[:, M1_R:M1_R + 2], in0=hpair(4, 3), in1=b2(h_(5)), op=OP.mult)
    V.tensor_tensor(out=smalls[:, M2_R:M2_R + 2], in0=hpair(3, 4), in1=hpair(2, 1), op=OP.mult)
    V.tensor_tensor(out=A9[:, 1:3], in0=smalls[:, M1_R:M1_R + 2],
                    in1=smalls[:, M2_R:M2_R + 2], op=OP.subtract)
    # c12 = h3 h4 - h0 h5  -> row 5
    V.tensor_tensor(out=smalls[:, T0_R], in0=h_(3), in1=h_(4), op=OP.mult)
    V.tensor_tensor(out=smalls[:, T1_R], in0=h_(0), in1=h_(5), op=OP.mult)
    V.tensor_tensor(out=A9[:, 5], in0=smalls[:, T0_R], in1=smalls[:, T1_R],
                    op=OP.subtract)

    # det = h0 c00 + h3 c01 + h4 c02 ; then symmetric dups {1,2}->{3,6}, {5}->{7}
    V.tensor_tensor(out=smalls[:, T0_R], in0=h_(0), in1=A9[:, 0], op=OP.mult)
    V.tensor_tensor(out=smalls[:, T1_R], in0=h_(3), in1=A9[:, 1], op=OP.mult)
    V.tensor_tensor(out=smalls[:, T2_R], in0=h_(4), in1=A9[:, 2], op=OP.mult)
    V.tensor_tensor(out=smalls[:, DET_R], in0=smalls[:, T0_R], in1=smalls[:, T1_R],
                    op=OP.add)
    V.tensor_tensor(out=smalls[:, DET_R], in0=smalls[:, DET_R], in1=smalls[:, T2_R],
                    op=OP.add)
    V.tensor_copy(out=A9[:, 3:7:3], in_=A9[:, 1:3])
    V.tensor_copy(out=A9[:, 7], in_=A9[:, 5])
    V.reciprocal(out=smalls[:, RDN_R], in_=smalls[:, DET_R])
    # A = adj * (-1/det), cast to bf16
    rdn_b = bc(smalls[:, RDN_R].unsqueeze(1), (P, 9, J))
    V.scalar_tensor_tensor(out=A9b[:], in0=A9[:], scalar=-1.0,
                           in1=rdn_b, op0=OP.mult, op1=OP.mult)

    # ---------------- bot = A @ Hth  (per output row c: rows 3c..3c+2 of P9)
    for c in range(3):
        V.tensor_tensor(out=P9[:, 3 * c:3 * c + 3],
                        in0=bc(A9b[:, 3 * c:3 * c + 3].unsqueeze(2), (P, 3, N_ACT, J)),
                        in1=HthT[:], op=OP.mult)
    P9g = P9[:].rearrange("p (c k) a j -> p c k a j", c=3)
    V.tensor_tensor(out=botT5[:, 0:3], in0=P9g[:, :, 0], in1=P9g[:, :, 1], op=OP.add)
    V.tensor_tensor(out=botT5[:, 0:3], in0=botT5[:, 0:3], in1=P9g[:, :, 2], op=OP.add)
    V.tensor_copy(out=botT5[:, 3:5], in_=botT5[:, 0:2])

    # bot -> outb cols 21:42 (ScalarE cast+transpose, overlaps top compute)
    S.copy(out=outb[:, :, 21:42].rearrange("p j (c a) -> p c a j", c=3),
           in_=botT5[:, 0:3])

    # ---------------- top = c x bot - jsm, per quarter; outb + DMA out
    for h in range(NCH):
        js = slice(h * CJ, (h + 1) * CJ)
        cq = lambda r0, r1: bc(cv5T[:, r0:r1, js].unsqueeze(2), (P, 3, N_ACT, CJ))
        V.tensor_tensor(out=ta[:, :, :, js], in0=cq(1, 4),
                        in1=botT5[:, 2:5, :, js], op=OP.mult)
        V.tensor_tensor(out=tb[:, :, :, js], in0=cq(2, 5),
                        in1=botT5[:, 1:4, :, js], op=OP.mult)
        V.tensor_tensor(out=ta[:, :, :, js], in0=ta[:, :, :, js],
                        in1=tb[:, :, :, js], op=OP.subtract)
        V.tensor_tensor(out=topT[:, :, :, js], in0=ta[:, :, :, js],
                        in1=jsmT[:, :, :, js], op=OP.subtract)
        S.copy(out=outb[:, js, 0:21].rearrange("p j (c a) -> p c a j", c=3),
               in_=topT[:, :, :, js])
        nc.sync.dma_start(out=dram["out"][:, js], in_=outb[:, js])


@functools.lru_cache(maxsize=1)
def _program():
    from contextlib import ExitStack
    import concourse.bacc as bacc
    import concourse.tile as tile
    from concourse import mybir

    f32 = mybir.dt.float32
    nc = bacc.Bacc("TRN2", target_bir_lowering=False, debug=False)
    dram = {
        "com": nc.dram_tensor("com", [P, J, 21], f32, kind="ExternalInput"),
        "pose": nc.dram_tensor("pose", [P, J, 144], f32, kind="ExternalInput"),
        "consts": nc.dram_tensor("consts", [P, NCONST], f32, kind="ExternalInput"),
        "out": nc.dram_tensor("out", [P, J, 42], f32, kind="ExternalOutput"),
    }
    with tile.TileContext(nc) as tc:
        with ExitStack() as ctx:
            _emit(nc, tc, ctx, dram)
    nc.compile()
    return nc


def _kernel_bm0(com, pose):
    # bm=0 path (not exercised by the shipped setup_inputs; numpy fallback)
    rot = pose[:, :, :3, 2, :N_ACT].copy()
    rot[..., 1] = pose[:, :, :3, 0, 1]
    rot[..., 5] = pose[:, :, :3, 0, 5]
    rot[..., 4] *= -1.0
    delp = pose[:, :, :3, 3, -2][..., None] - pose[:, :, :3, 3, :N_ACT]
    jt = np.cross(rot, delp, axis=2)
    return np.concatenate([jt, rot], axis=2).astype(np.float32)


def kernel(com_list, link_pose_list, bm):
    com_list = np.ascontiguousarray(com_list, dtype=np.float32)
    link_pose_list = np.ascontiguousarray(link_pose_list, dtype=np.float32)
    if not int(bm):
        return _kernel_bm0(com_list, link_pose_list)

    from concourse.bass_utils import run_bass_kernel_spmd

    nc = _program()
    com_flat = com_list.reshape(N_CORES, P, J, 21)
    pose_flat = link_pose_list.reshape(N_CORES, P, J, 144)
    consts = np.broadcast_to(CONSTS, (P, NCONST)).copy()
    in_maps = [
        {"com": com_flat[k], "pose": pose_flat[k], "consts": consts}
        for k in range(N_CORES)
    ]
    res = run_bass_kernel_spmd(nc, in_maps, core_ids=list(range(N_CORES)))
    out = np.stack([res.results[k]["out"] for k in range(N_CORES)])
    return out.reshape(512, 256, 6, 7)


# revision 5
# speedup vs baseline: 1.3867x; 1.0102x over previous
"""Trainium2 Bass kernel for nn_CanadarmJacob (space-arm Jacobian, bm=1 path).

Contract: kernel(**inputs) takes FULL inputs (com_list (512,256,3,7) f32,
link_pose_list (512,256,4,4,9) f32, bm scalar) and returns the FULL output
(512,256,6,7) f32. Internally shards samples across 8 NeuronCores (pure data
parallel), runs a Bass/Tile kernel per core, and gathers.

v2 design: J-innermost bf16 layout (P, row, act, J).
  - All bulk elementwise ops are bf16 tensor_tensor at the DVE 2x perf mode
    (innermost step 1, even runs). Per-sample scalars (c, A rows, 1/det)
    broadcast over the act dim with innermost step 1, staying at 2x.
  - ScalarE (Activation engine) does all input marshaling: transpose+cast of
    com/pose columns into the T layout, plus constant-tile materialization
    and the output cast/transpose back to (J, 42) fp32.
  - Act-dim reductions are shifted-add trees at 2x instead of 1x tensor_reduce.
  - Cross products use 5-row tiles (rows 3,4 duplicate 0,1) so rot[c+1] etc.
    are contiguous 3-row views -> 3 big TT ops instead of 9 small ones.
  - w2 = w - SM*c folds into the suffix cumsum: w2 = sufcum(mdel - M*c).
  - Pose DMA fetches only rows r<3 (432 of 576 B/sample).

Math (same reformulation as v1):
  pos   = pose[:3, 3, :7];  rot = pose[:3, AXIS[a], a], rot[:,4] *= -1
  del   = com - pos;  jacob = rot x del
  w2    = sufcum_a(M*del - M*c);  Hth = D_suf (.) rot + w2 x jacob
  S_cc' = sum_a M_a del_c del_c';  c = (sum_a M_a com_a)/TM - BASE
  H_s   = TM(c c^T - |c|^2 I) + CD I + SS I - S
  A     = -inv(H_s);  bot = A @ Hth
  top   = -(SM/TM) (.) jacob + c x bot
"""
import sys
import functools

if "/opt/trn_rl_repo" not in sys.path:
    sys.path.insert(0, "/opt/trn_rl_repo")

import numpy as np

# ---------------------------------------------------------------- constants
N_CORES = 8
P = 128          # SBUF partitions
J = 128          # samples per partition per core
NCH = 4          # pose DMA quarters
CJ = J // NCH
JH = J // 2
N_ACT = 7

MASS = np.array([105.98, 105.98, 314.98, 279.2, 105.98, 105.98, 243.66], np.float64)
TM = float(MASS.sum() + 100000.0 + 243.66)
DIAGS = np.array([[12.19, 12.19, 3.061], [12.19, 12.19, 3.061], [15.41, 2094.71, 2103.19],
                  [9.522, 1966.28, 1966.28], [8.305, 3.061, 8.0386], [12.13, 12.13, 3.061],
                  [9.336, 44.41, 44.41]], np.float64)
D_SUF = np.cumsum(DIAGS[::-1], axis=0)[::-1]          # (7,3) suffix inertia diag
SM = np.cumsum(MASS[::-1])[::-1]                      # (7,) suffix mass
CD = DIAGS.sum(axis=0)                                # (3,)
_TF0 = np.array([[1, 0, 0, 0], [0, -1, 0, 0], [0, 0, 1.3, 6], [0, 0, 0, 1]], np.float64)
_COM0 = np.array([[1, 0, 0, 0], [0, 1, 0, 0], [0, 0, 1, 0.5], [0, 0, 0, 1]], np.float64)
BASE = (_TF0 @ _COM0)[:3, 3] * 243.66 / (100000.0 + 243.66)   # [0, 0, ~0.0162]

# consts row layout (replicated to all 128 partitions host-side)
#   [0:7] M, [7:14] SM/TM, [14:35] D (c-major: D[c][a]), [35:38] CD
CONSTS = np.concatenate([MASS, SM / TM, D_SUF.T.reshape(-1), CD]).astype(np.float32)
NCONST = CONSTS.shape[0]

# smalls tile row indices (each row is (128, J) f32)
SS_R, CSQ_R = 0, 1
CC = 2            # rows 2..4 diag (xx,yy,zz), 5..7 off (xy,xz,yz)
HS = 8            # rows 8..13: [xx,yy,zz,xy,xz,yz]
M1_R, M2_R = 14, 16
T0_R, T1_R, T2_R = 18, 19, 20
DET_R, RDN_R = 21, 22
NSMALL = 23


def _emit(nc, tc, ctx, dram):
    from concourse import mybir

    f32 = mybir.dt.float32
    f16 = mybir.dt.float16
    OP = mybir.AluOpType
    V = nc.vector
    S = nc.scalar

    pool = ctx.enter_context(tc.tile_pool(name="main", bufs=1))
    ppool = ctx.enter_context(tc.tile_pool(name="pp", bufs=2))

    consts = pool.tile([P, NCONST], f32)
    com = pool.tile([P, J, 21], f32, tag="io")
    poses = [ppool.tile([P, CJ, 108], f32, tag="px", name=f"pose{h}")
             for h in range(NCH)]

    # T-layout tiles, J innermost (f16 unless noted)
    comT = pool.tile([P, 3, N_ACT, J], f16)
    delT5 = pool.tile([P, 5, N_ACT, J], f16, tag="five1")  # 0:3 del; 3,4 dup 0,1
    rotT5 = pool.tile([P, 5, N_ACT, J], f16)
    mdelT = pool.tile([P, 3, N_ACT, J], f16, tag="m37")
    prodT = pool.tile([P, 9, N_ACT, J], f16, tag="big9")  # 6 S + 3 mcom rows
    treeT = pool.tile([P, 9, 3, J], f16, tag="tmp9")  # reduction tree scratch
    red9 = pool.tile([P, 9, J], f32)            # act-sums: S(6), scom(3)
    w2T5 = pool.tile([P, 5, N_ACT, J], f16)
    jacT5 = pool.tile([P, 5, N_ACT, J], f16)
    HthT = pool.tile([P, 3, N_ACT, J], f16)
    jsmT = pool.tile([P, 3, N_ACT, J], f16)
    ta = pool.tile([P, 3, N_ACT, J], f16)
    tb = pool.tile([P, 3, N_ACT, J], f16)
    cvT = pool.tile([P, 3, J], f32)
    cv5T = pool.tile([P, 5, J], f16)
    smalls = pool.tile([P, NSMALL, J], f32)
    A9 = pool.tile([P, 9, J], f32, tag="tmp9")  # adjugate, row-major 3x3
    A9b = pool.tile([P, 9, J], f16)
    P9 = pool.tile([P, 9, N_ACT, J], f16, tag="big9")
    botT5 = pool.tile([P, 5, N_ACT, J], f16, tag="five1")
    outb = pool.tile([P, J, 42], f32, tag="io")

    # Tag-based overlap: HthT reuses comT's slot region? No -- keep separate
    # tags; pools pack by declaration. (SBUF fits: ~200KB/partition.)

    # ---------------- input DMAs: consts+com quarters on the ACT HWDGE ring,
    # pose quarters on the SP ring, so the two FIFO chains overlap.
    nc.scalar.dma_start(out=consts[:], in_=dram["consts"][:])
    for h in range(NCH):
        js = slice(h * CJ, (h + 1) * CJ)
        nc.scalar.dma_start(out=com[:, js], in_=dram["com"][:, js])
    for h in range(NCH):
        js = slice(h * CJ, (h + 1) * CJ)
        # rows r<3 of each 4x4x9 pose = first 108 of 144 floats per sample
        nc.sync.dma_start(out=poses[h][:], in_=dram["pose"][:, js, 0:108])

    def bc(ap, shape):
        return ap.broadcast_to(shape)

    # ---------------- ScalarE: materialize bf16 const tiles
    Mt = pool.tile([P, N_ACT, J], f16)
    SMTt = pool.tile([P, N_ACT, J], f16)
    Dt = pool.tile([P, 3, N_ACT, J], f16)
    S.copy(out=Mt[:], in_=bc(consts[:, 0:7].unsqueeze(2), (P, N_ACT, J)))
    S.copy(out=SMTt[:], in_=bc(consts[:, 7:14].unsqueeze(2), (P, N_ACT, J)))
    S.copy(out=Dt[:], in_=bc(
        consts[:, 14:35].rearrange("p (c a) -> p c a", c=3).unsqueeze(3),
        (P, 3, N_ACT, J)))
    Mb = bc(Mt[:].unsqueeze(1), (P, 3, N_ACT, J))
    SMTb = bc(SMTt[:].unsqueeze(1), (P, 3, N_ACT, J))

    # per-quarter marshal (ScalarE) + early compute (DVE)
    for h in range(NCH):
        js = slice(h * CJ, (h + 1) * CJ)
        # pose chunk viewed (p, j, r, k, l)
        pv = poses[h][:].rearrange("p j (r k l) -> p r k l j", r=3, k=4)
        # comT chunk: (p, c, a, j)
        S.copy(out=comT[:, :, :, js],
               in_=com[:, js, :].rearrange("p j (c a) -> p c a j", c=3))
        # pos -> delT5 rows 0:3 (holds pos until the sub)
        S.copy(out=delT5[:, 0:3, :, js], in_=pv[:, :, 3, 0:7])
        # rot main: axis col 2
        S.copy(out=rotT5[:, 0:3, :, js], in_=pv[:, :, 2, 0:7])
        # patch acts 1,5 from axis col 0
        S.copy(out=rotT5[:, 0:3, 1:6:4, js], in_=pv[:, :, 0, 1:6:4])
        # sign-flip act 4
        S.mul(rotT5[:, 0:3, 4, js], rotT5[:, 0:3, 4, js], -1.0)

        # --- DVE early stage (quarter)
        # del = com - pos  (in place over the pos copy)
        V.tensor_tensor(out=delT5[:, 0:3, :, js], in0=comT[:, :, :, js],
                        in1=delT5[:, 0:3, :, js], op=OP.subtract)
        # mdel = M * del
        V.tensor_tensor(out=mdelT[:, :, :, js], in0=Mb[:, :, :, js],
                        in1=delT5[:, 0:3, :, js], op=OP.mult)
        # S products: rows [S00,S11,S22, S01,S02, S12]
        V.tensor_tensor(out=prodT[:, 0:3, :, js], in0=mdelT[:, :, :, js],
                        in1=delT5[:, 0:3, :, js], op=OP.mult)
        V.tensor_tensor(out=prodT[:, 3:5, :, js],
                        in0=bc(mdelT[:, 0:1, :, js], (P, 2, N_ACT, CJ)),
                        in1=delT5[:, 1:3, :, js], op=OP.mult)
        V.tensor_tensor(out=prodT[:, 5, :, js], in0=mdelT[:, 1, :, js],
                        in1=delT5[:, 2, :, js], op=OP.mult)
        # mcom rows 6:9
        V.tensor_tensor(out=prodT[:, 6:9, :, js], in0=Mb[:, :, :, js],
                        in1=comT[:, :, :, js], op=OP.mult)
        # act-sum tree: sum7 = ((x0+x4)+(x1+x5)) + ((x2+x6)+x3)
        V.tensor_tensor(out=treeT[:, :, :, js], in0=prodT[:, :, 0:3, js],
                        in1=prodT[:, :, 4:7, js], op=OP.add)
        V.tensor_tensor(out=treeT[:, :, 0, js], in0=treeT[:, :, 0, js],
                        in1=treeT[:, :, 1, js], op=OP.add)
        V.tensor_tensor(out=treeT[:, :, 2, js], in0=treeT[:, :, 2, js],
                        in1=prodT[:, :, 3, js], op=OP.add)
        V.tensor_tensor(out=red9[:, :, js], in0=treeT[:, :, 0, js],
                        in1=treeT[:, :, 2, js], op=OP.add)

        # ---- mid stage per half, interleaved after quarters 1 and 3
        if h % 2 == 1:
            g = h // 2
            jh = slice(g * JH, (g + 1) * JH)
            # c = scom/TM - BASE  (fp32)
            V.tensor_scalar(out=cvT[:, 0:2, jh], in0=red9[:, 6:8, jh],
                            scalar1=1.0 / TM, scalar2=None, op0=OP.mult)
            V.tensor_scalar(out=cvT[:, 2, jh], in0=red9[:, 8, jh],
                            scalar1=1.0 / TM, scalar2=float(BASE[2]),
                            op0=OP.mult, op1=OP.subtract)
            # cv5 bf16 + dup rows
            V.tensor_copy(out=cv5T[:, 0:3, jh], in_=cvT[:, :, jh])
            V.tensor_copy(out=cv5T[:, 3:5, jh], in_=cv5T[:, 0:2, jh])
            # dups for shifted-row cross products
            V.tensor_copy(out=delT5[:, 3:5, :, jh], in_=delT5[:, 0:2, :, jh])
            V.tensor_copy(out=rotT5[:, 3:5, :, jh], in_=rotT5[:, 0:2, :, jh])
            cvb = bc(cv5T[:, 0:3, jh].unsqueeze(2), (P, 3, N_ACT, JH))
            # w2 = sufcum(mdel - M*c)
            V.tensor_tensor(out=ta[:, :, :, jh], in0=Mb[:, :, :, jh], in1=cvb,
                            op=OP.mult)
            V.tensor_tensor(out=w2T5[:, 0:3, :, jh], in0=mdelT[:, :, :, jh],
                            in1=ta[:, :, :, jh], op=OP.subtract)
            for k in range(5, -1, -1):
                V.tensor_tensor(out=w2T5[:, 0:3, k, jh],
                                in0=w2T5[:, 0:3, k, jh],
                                in1=w2T5[:, 0:3, k + 1, jh], op=OP.add)
            V.tensor_copy(out=w2T5[:, 3:5, :, jh], in_=w2T5[:, 0:2, :, jh])
            # jac = rot x del
            V.tensor_tensor(out=ta[:, :, :, jh], in0=rotT5[:, 1:4, :, jh],
                            in1=delT5[:, 2:5, :, jh], op=OP.mult)
            V.tensor_tensor(out=tb[:, :, :, jh], in0=rotT5[:, 2:5, :, jh],
                            in1=delT5[:, 1:4, :, jh], op=OP.mult)
            V.tensor_tensor(out=jacT5[:, 0:3, :, jh], in0=ta[:, :, :, jh],
                            in1=tb[:, :, :, jh], op=OP.subtract)
            V.tensor_copy(out=jacT5[:, 3:5, :, jh], in_=jacT5[:, 0:2, :, jh])
            # Hth = D*rot + w2 x jac
            V.tensor_tensor(out=ta[:, :, :, jh], in0=w2T5[:, 1:4, :, jh],
                            in1=jacT5[:, 2:5, :, jh], op=OP.mult)
            V.tensor_tensor(out=tb[:, :, :, jh], in0=w2T5[:, 2:5, :, jh],
                            in1=jacT5[:, 1:4, :, jh], op=OP.mult)
            V.tensor_tensor(out=HthT[:, :, :, jh], in0=ta[:, :, :, jh],
                            in1=tb[:, :, :, jh], op=OP.subtract)
            V.tensor_tensor(out=ta[:, :, :, jh], in0=Dt[:, :, :, jh],
                            in1=rotT5[:, 0:3, :, jh], op=OP.mult)
            V.tensor_tensor(out=HthT[:, :, :, jh], in0=HthT[:, :, :, jh],
                            in1=ta[:, :, :, jh], op=OP.add)
            # jsm = (SM/TM) * jac
            V.tensor_tensor(out=jsmT[:, :, :, jh], in0=SMTb[:, :, :, jh],
                            in1=jacT5[:, 0:3, :, jh], op=OP.mult)

    # ---------------- smalls chain (fp32, full J) -> A9b (f16)
    # cc products and |c|^2, SS
    V.tensor_tensor(out=smalls[:, CC:CC + 3], in0=cvT[:], in1=cvT[:], op=OP.mult)
    for k, (i, j) in enumerate([(0, 1), (0, 2), (1, 2)]):
        V.tensor_tensor(out=smalls[:, CC + 3 + k], in0=cvT[:, i], in1=cvT[:, j],
                        op=OP.mult)
    V.tensor_tensor(out=smalls[:, T0_R], in0=red9[:, 0], in1=red9[:, 1], op=OP.add)
    V.tensor_tensor(out=smalls[:, SS_R], in0=smalls[:, T0_R], in1=red9[:, 2],
                    op=OP.add)
    V.tensor_tensor(out=smalls[:, T0_R], in0=smalls[:, CC], in1=smalls[:, CC + 1],
                    op=OP.add)
    V.tensor_tensor(out=smalls[:, CSQ_R], in0=smalls[:, T0_R], in1=smalls[:, CC + 2],
                    op=OP.add)

    csq_b = bc(smalls[:, CSQ_R].unsqueeze(1), (P, 3, J))
    ss_b = bc(smalls[:, SS_R].unsqueeze(1), (P, 3, J))
    CDb = bc(consts[:, 35:38].unsqueeze(2), (P, 3, J))

    # H_s diag rows HS..HS+2 ; off rows HS+3..HS+5
    V.tensor_tensor(out=smalls[:, T0_R:T0_R + 3], in0=smalls[:, CC:CC + 3], in1=csq_b,
                    op=OP.subtract)
    V.tensor_tensor(out=smalls[:, HS:HS + 3], in0=ss_b, in1=red9[:, 0:3], op=OP.subtract)
    V.scalar_tensor_tensor(out=smalls[:, HS:HS + 3], in0=smalls[:, T0_R:T0_R + 3],
                           scalar=TM, in1=smalls[:, HS:HS + 3],
                           op0=OP.mult, op1=OP.add)
    V.tensor_tensor(out=smalls[:, HS:HS + 3], in0=smalls[:, HS:HS + 3], in1=CDb,
                    op=OP.add)
    V.scalar_tensor_tensor(out=smalls[:, HS + 3:HS + 6],
                           in0=smalls[:, CC + 3:CC + 6], scalar=TM,
                           in1=red9[:, 3:6], op0=OP.mult, op1=OP.subtract)

    # adjugate into A9 (row-major [c00,c01,c02, c10,c11,c12, c20,c21,c22];
    # unique cofactors first: c00->r0, c11->r4, c22->r8, c01->r1, c02->r2, c12->r5)
    h_ = lambda i: smalls[:, HS + i]
    hpair = lambda a, b: smalls[:, HS + a: (HS + b - 1 if b < a else HS + b + 1): (1 if b > a else -1)]
    b2 = lambda ap: bc(ap.unsqueeze(1), (P, 2, J))
    # c00 = h1 h2 - h5^2 ; c11 = h0 h2 - h4^2   -> rows {0,4}
    V.tensor_tensor(out=smalls[:, M1_R:M1_R + 2], in0=hpair(1, 0), in1=b2(h_(2)), op=OP.mult)
    V.tensor_tensor(out=smalls[:, M2_R:M2_R + 2], in0=hpair(5, 4), in1=hpair(5, 4), op=OP.mult)
    V.tensor_tensor(out=A9[:, 0:5:4], in0=smalls[:, M1_R:M1_R + 2],
                    in1=smalls[:, M2_R:M2_R + 2], op=OP.subtract)
    # c22 = h0 h1 - h3^2  -> row 8
    V.tensor_tensor(out=smalls[:, T0_R], in0=h_(0), in1=h_(1), op=OP.mult)
    V.tensor_tensor(out=smalls[:, T1_R], in0=h_(3), in1=h_(3), op=OP.mult)
    V.tensor_tensor(out=A9[:, 8], in0=smalls[:, T0_R], in1=smalls[:, T1_R],
                    op=OP.subtract)
    # c01 = h4 h5 - h3 h2 ; c02 = h3 h5 - h4 h1  -> rows {1,2}
    V.tensor_tensor(out=smalls[:, M1_R:M1_R + 2], in0=hpair(4, 3), in1=b2(h_(5)), op=OP.mult)
    V.tensor_tensor(out=smalls[:, M2_R:M2_R + 2], in0=hpair(3, 4), in1=hpair(2, 1), op=OP.mult)
    V.tensor_tensor(out=A9[:, 1:3], in0=smalls[:, M1_R:M1_R + 2],
                    in1=smalls[:, M2_R:M2_R + 2], op=OP.subtract)
    # c12 = h3 h4 - h0 h5  -> row 5
    V.tensor_tensor(out=smalls[:, T0_R], in0=h_(3), in1=h_(4), op=OP.mult)
    V.tensor_tensor(out=smalls[:, T1_R], in0=h_(0), in1=h_(5), op=OP.mult)
    V.tensor_tensor(out=A9[:, 5], in0=smalls[:, T0_R], in1=smalls[:, T1_R],
                    op=OP.subtract)

    # det = h0 c00 + h3 c01 + h4 c02 ; then symmetric dups {1,2}->{3,6}, {5}->{7}
    V.tensor_tensor(out=smalls[:, T0_R], in0=h_(0), in1=A9[:, 0], op=OP.mult)
    V.tensor_tensor(out=smalls[:, T1_R], in0=h_(3), in1=A9[:, 1], op=OP.mult)
    V.tensor_tensor(out=smalls[:, T2_R], in0=h_(4), in1=A9[:, 2], op=OP.mult)
    V.tensor_tensor(out=smalls[:, DET_R], in0=smalls[:, T0_R], in1=smalls[:, T1_R],
                    op=OP.add)
    V.tensor_tensor(out=smalls[:, DET_R], in0=smalls[:, DET_R], in1=smalls[:, T2_R],
                    op=OP.add)
    V.tensor_copy(out=A9[:, 3:7:3], in_=A9[:, 1:3])
    V.tensor_copy(out=A9[:, 7], in_=A9[:, 5])
    V.reciprocal(out=smalls[:, RDN_R], in_=smalls[:, DET_R])
    # A = adj * (-1/det), cast to bf16
    rdn_b = bc(smalls[:, RDN_R].unsqueeze(1), (P, 9, J))
    V.scalar_tensor_tensor(out=A9b[:], in0=A9[:], scalar=-1.0,
                           in1=rdn_b, op0=OP.mult, op1=OP.mult)

    # ---------------- bot = A @ Hth  (per output row c: rows 3c..3c+2 of P9)
    for c in range(3):
        V.tensor_tensor(out=P9[:, 3 * c:3 * c + 3],
                        in0=bc(A9b[:, 3 * c:3 * c + 3].unsqueeze(2), (P, 3, N_ACT, J)),
                        in1=HthT[:], op=OP.mult)
    P9g = P9[:].rearrange("p (c k) a j -> p c k a j", c=3)
    V.tensor_tensor(out=botT5[:, 0:3], in0=P9g[:, :, 0], in1=P9g[:, :, 1], op=OP.add)
    V.tensor_tensor(out=botT5[:, 0:3], in0=botT5[:, 0:3], in1=P9g[:, :, 2], op=OP.add)
    V.tensor_copy(out=botT5[:, 3:5], in_=botT5[:, 0:2])

    # bot -> outb cols 21:42 (ScalarE cast+transpose, overlaps top compute)
    S.copy(out=outb[:, :, 21:42].rearrange("p j (c a) -> p c a j", c=3),
           in_=botT5[:, 0:3])

    # ---------------- top = c x bot - jsm, per half; final sub writes outb
    # fp32 directly (1x) so no ScalarE cast sits on the tail; DMA out halves.
    for g in range(2):
        js = slice(g * JH, (g + 1) * JH)
        cq = lambda r0, r1: bc(cv5T[:, r0:r1, js].unsqueeze(2), (P, 3, N_ACT, JH))
        V.tensor_tensor(out=ta[:, :, :, js], in0=cq(1, 4),
                        in1=botT5[:, 2:5, :, js], op=OP.mult)
        V.tensor_tensor(out=tb[:, :, :, js], in0=cq(2, 5),
                        in1=botT5[:, 1:4, :, js], op=OP.mult)
        V.tensor_tensor(out=ta[:, :, :, js], in0=ta[:, :, :, js],
                        in1=tb[:, :, :, js], op=OP.subtract)
        V.tensor_tensor(
            out=outb[:, js, 0:21].rearrange("p j (c a) -> p c a j", c=3),
            in0=ta[:, :, :, js], in1=jsmT[:, :, :, js], op=OP.subtract)
        nc.sync.dma_start(out=dram["out"][:, js], in_=outb[:, js])


@functools.lru_cache(maxsize=1)
def _program():
    from contextlib import ExitStack
    import concourse.bacc as bacc
    import concourse.tile as tile
    from concourse import mybir

    f32 = mybir.dt.float32
    nc = bacc.Bacc("TRN2", target_bir_lowering=False, debug=False)
    dram = {
        "com": nc.dram_tensor("com", [P, J, 21], f32, kind="ExternalInput"),
        "pose": nc.dram_tensor("pose", [P, J, 144], f32, kind="ExternalInput"),
        "consts": nc.dram_tensor("consts", [P, NCONST], f32, kind="ExternalInput"),
        "out": nc.dram_tensor("out", [P, J, 42], f32, kind="ExternalOutput"),
    }
    with tile.TileContext(nc) as tc:
        with ExitStack() as ctx:
            _emit(nc, tc, ctx, dram)
    nc.compile()
    return nc


def _kernel_bm0(com, pose):
    # bm=0 path (not exercised by the shipped setup_inputs; numpy fallback)
    rot = pose[:, :, :3, 2, :N_ACT].copy()
    rot[..., 1] = pose[:, :, :3, 0, 1]
    rot[..., 5] = pose[:, :, :3, 0, 5]
    rot[..., 4] *= -1.0
    delp = pose[:, :, :3, 3, -2][..., None] - pose[:, :, :3, 3, :N_ACT]
    jt = np.cross(rot, delp, axis=2)
    return np.concatenate([jt, rot], axis=2).astype(np.float32)


def kernel(com_list, link_pose_list, bm):
    com_list = np.ascontiguousarray(com_list, dtype=np.float32)
    link_pose_list = np.ascontiguousarray(link_pose_list, dtype=np.float32)
    if not int(bm):
        return _kernel_bm0(com_list, link_pose_list)

    from concourse.bass_utils import run_bass_kernel_spmd

    nc = _program()
    com_flat = com_list.reshape(N_CORES, P, J, 21)
    pose_flat = link_pose_list.reshape(N_CORES, P, J, 144)
    consts = np.broadcast_to(CONSTS, (P, NCONST)).copy()
    in_maps = [
        {"com": com_flat[k], "pose": pose_flat[k], "consts": consts}
        for k in range(N_CORES)
    ]
    res = run_bass_kernel_spmd(nc, in_maps, core_ids=list(range(N_CORES)))
    out = np.stack([res.results[k]["out"] for k in range(N_CORES)])
    return out.reshape(512, 256, 6, 7)
